# revision 1
# baseline (speedup 1.0000x reference)
"""FuzzyPooling Trainium2 kernel.

Computes y = avgpool2x2(x * exp(-x^2/2)) for x of shape (32, 64, 224, 224) f32,
output (32, 64, 112, 112) f32.

Sharding: pure data parallel over the batch dim — core c takes x[4c:4c+4].

Per-core layout trick: with stride==kernel==2 pooling, each output row j of an
image comes from input rows 2j, 2j+1, which are contiguous in DRAM (448 floats).
So the per-core tensor (4*64*224*224 elems) is viewed as 28672 "row-pairs" of
448 contiguous floats.  A compute tile is 512 consecutive row-pairs laid out as
[128 partitions x 1792], partition p holding row-pairs 4p..4p+3 (7168 contiguous
bytes per partition -> clean large DMA descriptors).  The pooled output of a
tile is [128 x 448] and is exactly contiguous in the output tensor as well, so
both DMAs are pure reshapes of DRAM.

Per tile:
  ACT:  sq = Square(x);  e = Exp(-0.5*sq + ln(1/4))        (one table set)
  DVE:  m = x*e (in place over x);  v = m_evenrow + m_oddrow;
        o = v[::2] + v[1::2]
"""

import math

import numpy as np

import concourse.bass as bass  # noqa: F401  (bass types referenced via bacc/tile)
import concourse.mybir as mybir
from concourse import bacc, tile
from concourse.bass_utils import run_bass_kernel_spmd

N_CORES = 8
B, C, H, W = 32, 64, 224, 224
OH, OW = H // 2, W // 2
B_PER_CORE = B // N_CORES                      # 4
ROWPAIRS = B_PER_CORE * C * OH                 # 28672 row-pairs per core
RP_PER_PART = 4                                # row-pairs per partition per tile
TILE_RP = 128 * RP_PER_PART                    # 512 row-pairs per tile
N_TILES = ROWPAIRS // TILE_RP                  # 56
IN_FREE = RP_PER_PART * 2 * W                  # 1792 f32 per partition
OUT_FREE = RP_PER_PART * OW                    # 448 f32 per partition

_CACHE = {}


def emit_pass(nc, tc, x, out, bias, xpool, epool, vpool, opool):
    """Emit one full pass over the 56 per-core tiles."""
    f32 = mybir.dt.float32
    for t in range(N_TILES):
        xt = xpool.tile([128, IN_FREE], f32, tag="xt")
        nc.sync.dma_start(out=xt[:], in_=x[t])
        sq = epool.tile([128, IN_FREE], f32, tag="sq")
        if t % 4 == 3:
            # every 4th tile: square on DVE to balance ACT vs DVE
            # sq = (x * -0.5) * x = -x^2/2
            nc.vector.scalar_tensor_tensor(
                out=sq[:], in0=xt[:], scalar=-0.5, in1=xt[:],
                op0=mybir.AluOpType.mult, op1=mybir.AluOpType.mult)
            exp_scale = 1.0
        else:
            nc.scalar.activation(sq[:], xt[:],
                                 mybir.ActivationFunctionType.Square)
            exp_scale = -0.5
        # e = exp(-0.5*x^2 + ln(1/4)) = exp(-x^2/2)/4   (in place)
        nc.scalar.activation(sq[:], sq[:],
                             mybir.ActivationFunctionType.Exp,
                             bias=bias[:], scale=exp_scale)
        # m = x * e   (in place over the input tile)
        nc.vector.tensor_mul(out=xt[:], in0=xt[:], in1=sq[:])
        xv = xt[:].rearrange("p (k t w) -> p k t w", k=RP_PER_PART, t=2)
        v = vpool.tile([128, RP_PER_PART * W], f32, tag="v")
        # vertical (row-pair) add; DVE beats GPSIMD here on HW (~28us/pass)
        nc.vector.tensor_tensor(
            out=v[:].rearrange("p (k w) -> p k w", k=RP_PER_PART),
            in0=xv[:, :, 0, :], in1=xv[:, :, 1, :],
            op=mybir.AluOpType.add)
        vp = v[:].rearrange("p (k w t) -> p k w t", k=RP_PER_PART, t=2)
        o = opool.tile([128, OUT_FREE], f32, tag="o")
        nc.vector.tensor_add(
            out=o[:].rearrange("p (k w) -> p k w", k=RP_PER_PART),
            in0=vp[:, :, :, 0], in1=vp[:, :, :, 1])
        nc.scalar.dma_start(out=out[t], in_=o[:])


def _build_nc():
    f32 = mybir.dt.float32
    nc = bacc.Bacc("TRN2", target_bir_lowering=False, debug=False,
                   num_devices=N_CORES)
    x = nc.dram_tensor("x", [N_TILES, 128, IN_FREE], f32,
                       kind="ExternalInput").ap()
    out = nc.dram_tensor("out", [N_TILES, 128, OUT_FREE], f32,
                         kind="ExternalOutput").ap()

    with tile.TileContext(nc) as tc:
        with tc.tile_pool(name="const", bufs=1) as cpool, \
             tc.tile_pool(name="xin", bufs=7) as xpool, \
             tc.tile_pool(name="e", bufs=7) as epool, \
             tc.tile_pool(name="v", bufs=7) as vpool, \
             tc.tile_pool(name="o", bufs=8) as opool:
            bias = cpool.tile([128, 1], f32)
            nc.vector.memset(bias[:], math.log(0.25))
            emit_pass(nc, tc, x, out, bias, xpool, epool, vpool, opool)
    nc.compile()
    return nc


def _get_nc():
    if "nc" not in _CACHE:
        _CACHE["nc"] = _build_nc()
    return _CACHE["nc"]


def _run(x: np.ndarray, trace: bool = False):
    nc = _get_nc()
    in_maps = []
    for c in range(N_CORES):
        shard = np.ascontiguousarray(x[c * B_PER_CORE:(c + 1) * B_PER_CORE])
        in_maps.append({"x": shard.reshape(N_TILES, 128, IN_FREE)})
    res = run_bass_kernel_spmd(nc, in_maps, core_ids=list(range(N_CORES)),
                               trace=trace)
    parts = [r["out"].reshape(B_PER_CORE, C, OH, OW) for r in res.results]
    return np.concatenate(parts, axis=0), res


def kernel(x: np.ndarray) -> np.ndarray:
    out, _ = _run(np.asarray(x, dtype=np.float32), trace=False)
    return out



# revision 2
# speedup vs baseline: 1.5025x; 1.5025x over previous
"""FuzzyPooling Trainium2 kernel.

Computes y = avgpool2x2(x * exp(-x^2/2)) for x of shape (32, 64, 224, 224) f32,
output (32, 64, 112, 112) f32.

Sharding: pure data parallel over the batch dim — core c takes x[4c:4c+4].

Layout: with stride==kernel==2 pooling, each output row j of an image comes
from input rows 2j, 2j+1 — 448 contiguous floats in DRAM ("row-pair").  The
per-core tensor (4*64*224*224 elems) is 28672 row-pairs; a DMA chunk is
[128 partitions x 16 row-pairs] = [128 x 7168] f32 (28 KiB contiguous per
partition, 3.67 MB per transfer, 14 chunks/pass), and the pooled output chunk
[128 x 1792] f32 is exactly contiguous in the output tensor too — both DMAs
are pure reshapes of DRAM.

Math: exp(-x^2/2) = (sqrt(pi)/2) * d/dx erf(x/sqrt(2)), so the ACT engine's
Derivative_Erf table computes the gaussian in ONE pass (no Square pass), and
the constant (sqrt(pi)/2)*(1/4 pool mean) = sqrt(pi)/8 folds into the DVE
multiply.

Engine budget per core per pass (measured ~196 us, DMA-bound):
  DMA:  51.4 MB in + 12.85 MB out = 64.25 MB @ ~330 GB/s  (~195 us; the
        dma-only floor for this pattern measures 195-198 us vs 179 us HBM cap)
  in-DMA is SWDGE (gpsimd) with f32->bf16 cast-on-transfer; out-DMA HWDGE.
  ACT:  Derivative_Erf over 12.85M elems   ~88 us
  DVE (bf16, 2x/cycle tensor_tensor):  m=(e*K)*x, row-add, col-add  ~110 us
Accuracy: bf16 intermediates give rel_err ~3.4e-3 (vs 2e-2 gate).
"""

import math

import numpy as np

import concourse.bass as bass  # noqa: F401
import concourse.mybir as mybir
from concourse import bacc, tile
from concourse.bass_utils import run_bass_kernel_spmd

AF = mybir.ActivationFunctionType
ALU = mybir.AluOpType

N_CORES = 8
B, C, H, W = 32, 64, 224, 224
OH, OW = H // 2, W // 2
B_PER_CORE = B // N_CORES              # 4
ROWPAIRS = B_PER_CORE * C * OH         # 28672 row-pairs of 448 f32 per core
RP_C = 16                              # row-pairs per partition per DMA chunk
IN_F = RP_C * 2 * W                    # 7168 f32 per partition per chunk
OUT_F = RP_C * OW                      # 1792 f32
NCHUNK = ROWPAIRS // (128 * RP_C)      # 14
K = math.sqrt(math.pi) / 8.0           # (sqrt(pi)/2) [dErf] * (1/4) [mean]
S2 = 1.0 / math.sqrt(2.0)
BUFS = (4, 4, 4, 4)

_CACHE = {}


def _emit_chunk(nc, ch, x, out, pools):
    f32, bf16 = mybir.dt.float32, mybir.dt.bfloat16
    xpool, epool, vpool, opool = pools
    xt = xpool.tile([128, IN_F], bf16, tag="xt")
    nc.gpsimd.dma_start(out=xt[:], in_=x[ch])   # SWDGE: cast f32->bf16 on DMA
    et = epool.tile([128, IN_F], bf16, tag="et")
    # e = dErf(x/sqrt2) = (2/sqrt(pi)) exp(-x^2/2)
    nc.scalar.activation(et[:], xt[:], AF.Derivative_Erf, scale=S2)
    # m = (e * K) * x = x exp(-x^2/2) / 4   (in place over et)
    nc.vector.scalar_tensor_tensor(out=et[:], in0=et[:], scalar=K, in1=xt[:],
                                   op0=ALU.mult, op1=ALU.mult)
    mv = et[:].rearrange("p (k t w) -> p k t w", k=RP_C, t=2)
    v = vpool.tile([128, IN_F // 2], bf16, tag="v")
    vv = v[:].rearrange("p (k w) -> p k w", k=RP_C)
    nc.vector.tensor_tensor(out=vv, in0=mv[:, :, 0, :], in1=mv[:, :, 1, :],
                            op=ALU.add)
    vp = v[:].rearrange("p (k w t) -> p k w t", k=RP_C, t=2)
    o = opool.tile([128, OUT_F], f32, tag="o")
    ov = o[:].rearrange("p (k w) -> p k w", k=RP_C)
    nc.vector.tensor_tensor(out=ov, in0=vp[:, :, :, 0], in1=vp[:, :, :, 1],
                            op=ALU.add)
    nc.scalar.dma_start(out=out[ch], in_=o[:])


def _build_nc():
    f32 = mybir.dt.float32
    nc = bacc.Bacc("TRN2", target_bir_lowering=False, debug=False,
                   num_devices=N_CORES)
    x = nc.dram_tensor("x", [NCHUNK, 128, IN_F], f32,
                       kind="ExternalInput").ap()
    out = nc.dram_tensor("out", [NCHUNK, 128, OUT_F], f32,
                         kind="ExternalOutput").ap()
    with tile.TileContext(nc) as tc:
        with tc.tile_pool(name="xin", bufs=BUFS[0]) as xpool, \
             tc.tile_pool(name="e", bufs=BUFS[1]) as epool, \
             tc.tile_pool(name="v", bufs=BUFS[2]) as vpool, \
             tc.tile_pool(name="o", bufs=BUFS[3]) as opool:
            pools = (xpool, epool, vpool, opool)
            for ch in range(NCHUNK):
                _emit_chunk(nc, ch, x, out, pools)
    nc.compile()
    return nc


def _get_nc():
    if "nc" not in _CACHE:
        _CACHE["nc"] = _build_nc()
    return _CACHE["nc"]


def _run(x: np.ndarray, trace: bool = False):
    nc = _get_nc()
    in_maps = []
    for c in range(N_CORES):
        shard = np.ascontiguousarray(x[c * B_PER_CORE:(c + 1) * B_PER_CORE])
        in_maps.append({"x": shard.reshape(NCHUNK, 128, IN_F)})
    res = run_bass_kernel_spmd(nc, in_maps, core_ids=list(range(N_CORES)),
                               trace=trace)
    parts = [r["out"].reshape(B_PER_CORE, C, OH, OW) for r in res.results]
    return np.concatenate(parts, axis=0), res


def kernel(x: np.ndarray) -> np.ndarray:
    out, _ = _run(np.asarray(x, dtype=np.float32), trace=False)
    return out
